# revision 1
# baseline (speedup 1.0000x reference)
"""AttentionBlock (GroupNorm + single-head self-attention + residual) on 8 trn2 cores.

Sharding: core = (batch b = core//2, token-half h = core%2).  Each core gets the
full (128, 4096) channel-major image for its batch (needed for groupnorm stats
and full K/V), computes attention only for its 2048-token half, and writes a
(128, 2048) output slab.  The host rolls the token axis per-core so the q-half
is always columns [0:2048] -> one SPMD program for all 8 cores, no collectives.

Algebraic restructure (all exact, folded on the host):
  scoresT[m,n] = k_m . q_n = hn_m^T (Wk^T Wq') hn_n  (+ a per-n constant that
  cancels in softmax; Wq' absorbs the 1/sqrt(c) scale).  With M := Wk^T Wq'
  the score matmuls contract hn tiles against q2 = M.hn -- K is never
  materialized.  The k-bias shifts all scores of a q-column equally and drops
  out of softmax.  (Nonzero q-bias adds a per-m term: slow-path build flag.)

  attn out: y = x + Wp.(V.e)/den + bp,  V = Wv.hn + bv
          = x + (Wp.Wv).(hn.e)/den + (bp + Wp.bv)
  so the attn.V matmul uses plain PE *transposes* of hn tiles, and Wv/bv fold
  into the host-side proj weight W2 = Wp.Wv and bias bp2 = bp + Wp.bv.

Per core (c = channels on partitions):
  groupnorm stats via bn_stats + two tiny group-mask matmuls (cross-partition)
  hn = alpha*x + beta (DVE); q2 = M.hn (PE); hnT tiles via PE transpose
  per q-block (512) x k-triple (3x128):
      scoresT = hn_tile^T q2_blk  (PE, f32r, PSUM)
      attnT   = exp(scoresT)      (ACT, PSUM->SBUF, 1536-wide instr)
      out    += hnT_tile^T attnT  (PE, f32r, PSUM accumulate)
      den    += 1^T attnT         (PE ones-matmul / DVE adds, split 40/88)
  y = x + W2.(out) * (1/den broadcast) + bp2

float32r everywhere on the PE: full fp32 data, 1 cycle/row at N>=256
(plain float32 matmul streams at 1/4 rate on trn2).
"""

import numpy as np

C = 128        # channels
N = 4096       # tokens per batch (64*64)
NQ = 2048      # q tokens per core
B = 4
NCORES = 8
GROUPS = 8
EPS = 1e-5
QB = 512       # q block (one PSUM bank of fp32)
NQB = NQ // QB # 4
KT = 128       # k tile (partition dim)
NKT = N // KT  # 32
KT_GROUPS = [3] * 10 + [2]   # k-tile triples (fewer ACT overheads)

_CACHE = {}


def _den_group_on_pe(gi):
    # 4 of 11 k-tile groups' denominator on PE, rest fused on DVE
    return gi % 3 == 0


def _build_nc(repeat=1, with_qbias=False):
    from contextlib import ExitStack

    import concourse.bacc as bacc
    import concourse.bass as bass
    import concourse.mybir as mybir
    import concourse.tile as tile
    from concourse.mybir import ActivationFunctionType as AF
    from concourse.mybir import AluOpType as ALU

    fp32 = mybir.dt.float32
    f32r = mybir.dt.float32r

    nc = bacc.Bacc()

    x_d = nc.dram_tensor("x", [C, N], fp32, kind="ExternalInput")
    mqk_d = nc.dram_tensor("mqk", [C, C], f32r, kind="ExternalInput")
    w2t_d = nc.dram_tensor("w2t", [C, C], f32r, kind="ExternalInput")
    ident_d = nc.dram_tensor("ident", [C, C], f32r, kind="ExternalInput")
    uq_d = nc.dram_tensor("uq", [C, 1], f32r, kind="ExternalInput")
    bp2_d = nc.dram_tensor("bp2", [C, 1], fp32, kind="ExternalInput")
    gsc_d = nc.dram_tensor("gscale", [C, 1], fp32, kind="ExternalInput")
    gbi_d = nc.dram_tensor("gbias", [C, 1], fp32, kind="ExternalInput")
    mka_d = nc.dram_tensor("maska", [C, GROUPS], fp32, kind="ExternalInput")
    mkb_d = nc.dram_tensor("maskb", [GROUPS, C], fp32, kind="ExternalInput")
    y_d = nc.dram_tensor("y", [C, NQ], fp32, kind="ExternalOutput")

    with tile.TileContext(nc) as tc, ExitStack() as ctx:
        const = ctx.enter_context(tc.tile_pool(name="const", bufs=1))
        big = ctx.enter_context(tc.tile_pool(name="big", bufs=1))
        small = ctx.enter_context(tc.tile_pool(name="small", bufs=1))

        # small consts on the sync queue (ahead of x), matrices on gpsimd's
        bp2_sb = const.tile([C, 1], fp32)
        nc.sync.dma_start(out=bp2_sb, in_=bp2_d[:, :])
        gsc_sb = const.tile([C, 1], fp32)
        nc.sync.dma_start(out=gsc_sb, in_=gsc_d[:, :])
        gbi_sb = const.tile([C, 1], fp32)
        nc.sync.dma_start(out=gbi_sb, in_=gbi_d[:, :])
        mka_sb = const.tile([C, GROUPS], fp32)
        nc.sync.dma_start(out=mka_sb, in_=mka_d[:, :])
        mkb_sb = const.tile([GROUPS, C], fp32)
        nc.sync.dma_start(out=mkb_sb, in_=mkb_d[:, :])
        uq_sb = const.tile([C, 1], f32r)
        nc.sync.dma_start(out=uq_sb, in_=uq_d[:, :])
        mqk_sb = const.tile([C, C], f32r)
        nc.gpsimd.dma_start(out=mqk_sb, in_=mqk_d[:, :])
        w2t_sb = const.tile([C, C], f32r)
        nc.gpsimd.dma_start(out=w2t_sb, in_=w2t_d[:, :])
        ident_sb = const.tile([C, C], f32r)
        nc.gpsimd.dma_start(out=ident_sb, in_=ident_d[:, :])

        ones_col = const.tile([C, 1], fp32)
        nc.vector.memset(ones_col, 1.0)
        ones_col_r = const.tile([C, 1], f32r)
        nc.vector.tensor_copy(ones_col_r, ones_col)
        ones_row_r = const.tile([1, QB], f32r)
        nc.vector.memset(ones_row_r.bitcast(mybir.dt.uint32), 0x3F800000)
        eps_sb = const.tile([C, 1], fp32)
        nc.vector.memset(eps_sb, EPS)
        warm_sb = const.tile([1, 1], fp32)
        nc.scalar.activation(warm_sb, eps_sb[0:1, :], AF.Exp, bias=0.0, scale=0.0)

        rep_ctx = tc.For_i(0, repeat, 1) if repeat > 1 else None
        if rep_ctx is not None:
            rep_ctx.__enter__()

        x_sb = big.tile([C, N], fp32, tag="x")
        for ci in range(8):
            eng = nc.sync if ci % 2 == 0 else nc.scalar
            eng.dma_start(
                out=x_sb[:, ci * 512 : (ci + 1) * 512],
                in_=x_d[:, ci * 512 : (ci + 1) * 512],
            )

        # ---- groupnorm stats ----
        NCHUNK = N // 512
        with tc.tile_pool(name="stat_ps", bufs=2, space="PSUM") as stat_ps:
            stats = small.tile([C, NCHUNK, 6], fp32)
            for i in range(NCHUNK):
                nc.vector.bn_stats(
                    out=stats[:, i, :], in_=x_sb[:, i * 512 : (i + 1) * 512]
                )
            mv = small.tile([C, 2], fp32)
            nc.vector.bn_aggr(out=mv, in_=stats)

            # S = [m, v + m^2] per channel
            S = small.tile([C, 2], fp32)
            nc.vector.tensor_copy(S[:, 0:1], mv[:, 0:1])
            msq = small.tile([C, 1], fp32)
            nc.vector.tensor_mul(msq, mv[:, 0:1], mv[:, 0:1])
            nc.vector.tensor_add(S[:, 1:2], mv[:, 1:2], msq)

            # group-reduce across partitions via mask matmuls
            g_ps = stat_ps.tile([GROUPS, 2], fp32)
            nc.tensor.matmul(g_ps, mka_sb, S, start=True, stop=True)
            g_sb = small.tile([GROUPS, 2], fp32)
            nc.vector.tensor_copy(g_sb, g_ps)
            g2_ps = stat_ps.tile([C, 2], fp32)
            nc.tensor.matmul(g2_ps, mkb_sb, g_sb, start=True, stop=True)

            gsz = C // GROUPS
            mean_g = small.tile([C, 1], fp32)
            nc.vector.tensor_scalar_mul(mean_g, g2_ps[:, 0:1], 1.0 / gsz)
            e2_g = small.tile([C, 1], fp32)
            nc.vector.tensor_scalar_mul(e2_g, g2_ps[:, 1:2], 1.0 / gsz)
            var_g = small.tile([C, 1], fp32)
            nc.vector.tensor_mul(var_g, mean_g, mean_g)
            nc.vector.tensor_tensor(out=var_g, in0=e2_g, in1=var_g, op=ALU.subtract)
            # rstd = exp(-0.5*ln(var+eps)) -- stays on the exp table set
            lnv_g = small.tile([C, 1], fp32)
            nc.scalar.activation(lnv_g, var_g, AF.Ln, bias=eps_sb, scale=1.0)
            rstd_g = small.tile([C, 1], fp32)
            nc.scalar.activation(rstd_g, lnv_g, AF.Exp, bias=0.0, scale=-0.5)
            alpha = small.tile([C, 1], fp32)
            nc.vector.tensor_mul(alpha, rstd_g, gsc_sb)
            beta = small.tile([C, 1], fp32)
            nc.vector.tensor_mul(beta, mean_g, alpha)
            nc.vector.tensor_tensor(out=beta, in0=gbi_sb, in1=beta, op=ALU.subtract)

        # ---- hn = alpha*x + beta (DVE, chunked) ----
        hn = big.tile([C, N], f32r, tag="hn")
        for ci in range(4):
            nc.vector.tensor_scalar(
                out=hn[:, ci * 1024 : (ci + 1) * 1024],
                in0=x_sb[:, ci * 1024 : (ci + 1) * 1024],
                scalar1=alpha, scalar2=beta, op0=ALU.mult, op1=ALU.add,
            )

        # ---- q2 = M.hn (q half), hnT tiles (full) ----
        q2_sb = big.tile([C, NQB, QB], f32r, tag="q2")
        hnT_sb = big.tile([KT, NKT, C], f32r, tag="hnT")
        gam_sb = None
        if with_qbias:
            gam_sb = big.tile([1, N], f32r, tag="gam")

        with (
            tc.tile_pool(name="qk_ps", bufs=2, space="PSUM") as qk_ps,
            tc.tile_pool(name="v_ps", bufs=2, space="PSUM") as v_ps,
        ):
            for j in range(NQB * 2):
                vp = v_ps.tile([KT, 4, C], fp32, tag="v")
                for t in range(4):
                    kt = j * 4 + t
                    nc.tensor.matmul(
                        vp[:, t, :], hn[:, kt * KT : (kt + 1) * KT], ident_sb,
                        start=True, stop=True,
                    )
                nc.vector.tensor_copy(hnT_sb[:, j * 4 : (j + 1) * 4, :], vp)
                if j >= NQB:
                    continue
                ps2 = qk_ps.tile([C, 512], fp32, tag="qk")
                nc.tensor.matmul(
                    ps2, mqk_sb, hn[:, j * 512 : (j + 1) * 512],
                    start=True, stop=True,
                )
                nc.vector.tensor_copy(q2_sb[:, j, :], ps2)
            if with_qbias:
                for j in range(N // 512):
                    gp = qk_ps.tile([1, 512], fp32, tag="qg")
                    nc.tensor.matmul(
                        gp, uq_sb, hn[:, j * 512 : (j + 1) * 512],
                        start=True, stop=True,
                    )
                    nc.vector.tensor_copy(
                        gam_sb[:, j * 512 : (j + 1) * 512], gp
                    )

        # ---- attention main loop ----
        aout_sb = big.tile([C, NQB, QB], f32r, tag="aout")
        rden_dram = nc.dram_tensor("rden_scratch", [NQB, QB], fp32, kind="Internal")
        with (
            tc.tile_pool(name="s_ps", bufs=2, space="PSUM") as spool,
            tc.tile_pool(name="o_ps", bufs=1, space="PSUM") as opool,
            tc.tile_pool(name="d_ps", bufs=1, space="PSUM") as dpool,
            tc.tile_pool(name="attn", bufs=3) as apool,
        ):
            def emit_y(pend):
                pp_sb_, rbc_, qb_ = pend
                y_sb = small.tile([C, QB], fp32, tag="y", bufs=2)
                nc.vector.tensor_mul(y_sb, pp_sb_, rbc_)
                nc.vector.tensor_add(
                    y_sb, y_sb, x_sb[:, qb_ * QB : (qb_ + 1) * QB]
                )
                nc.vector.tensor_scalar_add(y_sb, y_sb, bp2_sb)
                nc.sync.dma_start(out=y_d[:, qb_ * QB : (qb_ + 1) * QB], in_=y_sb)

            pending_y = None
            for qb in range(NQB):
                out_ps = opool.tile([C, QB], fp32, tag="out")
                den_ps = dpool.tile([1, QB], fp32, tag="den")
                den_sb = small.tile([KT, 3, QB], f32r, tag="densb", bufs=2)
                qv = q2_sb[:, qb, :]
                kt = 0
                dve_den_started = False
                pe_den_started = False
                for gi, gsize in enumerate(KT_GROUPS):
                    s_ps = spool.tile([KT, 3, QB], fp32, tag="s")
                    for t in range(gsize):
                        nc.tensor.matmul(
                            s_ps[:, t, :],
                            hn[:, (kt + t) * KT : (kt + t + 1) * KT],
                            qv,
                            start=True,
                            stop=(not with_qbias),
                        )
                        if with_qbias:
                            # += gamma[m] broadcast along q (ones row rhs)
                            nc.tensor.matmul(
                                s_ps[:, t, :],
                                gam_sb[:, (kt + t) * KT : (kt + t + 1) * KT],
                                ones_row_r,
                                start=False,
                                stop=True,
                            )
                    at = apool.tile([KT, 3, QB], f32r, tag="at")
                    nc.scalar.activation(at[:, :gsize, :], s_ps[:, :gsize, :], AF.Exp)
                    if gi == 2 and pending_y is not None:
                        emit_y(pending_y)
                        pending_y = None
                    for t in range(gsize):
                        k_idx = kt + t
                        nc.tensor.matmul(
                            out_ps,
                            hnT_sb[:, k_idx, :],
                            at[:, t, :],
                            start=(k_idx == 0),
                            stop=(k_idx == NKT - 1),
                        )
                        if _den_group_on_pe(gi):
                            nc.tensor.matmul(
                                den_ps,
                                ones_col_r,
                                at[:, t, :],
                                start=(not pe_den_started),
                                stop=False,
                            )
                            pe_den_started = True
                    if not _den_group_on_pe(gi):
                        # fused 1024-wide denominator accumulate on DVE
                        if not dve_den_started:
                            nc.vector.tensor_copy(
                                den_sb[:, :gsize, :], at[:, :gsize, :]
                            )
                            dve_den_started = True
                        else:
                            nc.vector.tensor_add(
                                den_sb[:, :gsize, :], den_sb[:, :gsize, :],
                                at[:, :gsize, :],
                            )
                    kt += gsize
                # fold the DVE part into den_ps (completes the accumulation)
                nc.tensor.matmul(den_ps, ones_col_r, den_sb[:, 0, :],
                                 start=False, stop=False)
                nc.tensor.matmul(den_ps, ones_col_r, den_sb[:, 1, :],
                                 start=False, stop=False)
                nc.tensor.matmul(den_ps, ones_col_r, den_sb[:, 2, :],
                                 start=False, stop=True)

                # copy unnormalized accumulator out early (frees out_ps without
                # waiting on the denominator chain)
                nc.vector.tensor_copy(aout_sb[:, qb, :], out_ps)

                # denominator chain: recip -> DRAM-bounce partition broadcast
                rden = small.tile([1, QB], fp32, tag="rden", bufs=2)
                nc.vector.reciprocal(rden, den_ps)
                nc.sync.dma_start(out=rden_dram[qb : qb + 1, :], in_=rden)
                rbc = small.tile([C, QB], fp32, tag="rbc", bufs=2)
                rd_ap = rden_dram[qb : qb + 1, :]
                nc.sync.dma_start(
                    out=rbc,
                    in_=bass.AP(
                        tensor=rd_ap.tensor, offset=rd_ap.offset, ap=[[0, C], [1, QB]]
                    ),
                )

                # proj on the unnormalized accumulator; the rbc-gated y chain is
                # deferred into the next q-block's pipeline (hides the DMA
                # broadcast latency and frees slots early)
                pp = opool.tile([C, QB], fp32, tag="out")
                nc.tensor.matmul(
                    pp, w2t_sb, aout_sb[:, qb, :], start=True, stop=True
                )
                pp_sb = small.tile([C, QB], fp32, tag="ppsb", bufs=2)
                nc.vector.tensor_copy(pp_sb, pp)
                pending_y = (pp_sb, rbc, qb)

            if pending_y is not None:
                emit_y(pending_y)

        if rep_ctx is not None:
            rep_ctx.__exit__(None, None, None)

    nc.compile()
    return nc


def _prep_maps(x):
    x = np.ascontiguousarray(np.asarray(x, dtype=np.float32))
    b, c, h, w = x.shape
    assert (b, c, h * w) == (B, C, N), f"unexpected shape {x.shape}"
    return x.reshape(b, c, h * w)


def _make_in_maps(x, norm_scale, norm_bias, wq, bq, wk, bk, wv, bv, wp, bp):
    xr = _prep_maps(x)
    s = float(C) ** -0.5
    f32 = np.float32
    f64 = np.float64

    wqs = np.asarray(wq, f64) * s
    wk64 = np.asarray(wk, f64)
    wv64 = np.asarray(wv, f64)
    wp64 = np.asarray(wp, f64)
    bq64 = np.asarray(bq, f64) * s
    bv64 = np.asarray(bv, f64)
    bp64 = np.asarray(bp, f64)

    # scores: hn^T (Wk^T Wq') hn ; lhsT for q2 = M.hn is M^T = Wq'^T Wk
    mqk = np.ascontiguousarray((wqs.T @ wk64).astype(f32))
    # proj: W2 = Wp.Wv, lhsT = W2^T ; bias bp2 = bp + Wp.bv
    w2t = np.ascontiguousarray((wp64 @ wv64).T.astype(f32))
    bp2 = np.ascontiguousarray((bp64 + wp64 @ bv64).astype(f32).reshape(C, 1))
    # q-bias term (slow path only): u = Wk^T bq'
    uq = np.ascontiguousarray((wk64.T @ bq64).astype(f32).reshape(C, 1))
    ident = np.ascontiguousarray(np.eye(C, dtype=f32))
    gsc = np.ascontiguousarray(np.asarray(norm_scale, f32).reshape(C, 1))
    gbi = np.ascontiguousarray(np.asarray(norm_bias, f32).reshape(C, 1))
    maska = np.zeros((C, GROUPS), f32)
    maska[np.arange(C), np.arange(C) // (C // GROUPS)] = 1.0
    maskb = np.ascontiguousarray(maska.T)

    with_qbias = bool(np.any(np.asarray(bq) != 0))

    in_maps = []
    for core in range(NCORES):
        bi, hi = core // 2, core % 2
        xb = xr[bi]
        if hi:
            xb = np.roll(xb, -NQ, axis=1)
        in_maps.append(
            dict(
                x=np.ascontiguousarray(xb),
                mqk=mqk, w2t=w2t, ident=ident, uq=uq, bp2=bp2,
                gscale=gsc, gbias=gbi, maska=maska, maskb=maskb,
            )
        )
    return in_maps, with_qbias


def kernel(x, norm_scale, norm_bias, wq, bq, wk, bk, wv, bv, wp, bp):
    from concourse.bass_utils import run_bass_kernel_spmd

    in_maps, with_qbias = _make_in_maps(
        x, norm_scale, norm_bias, wq, bq, wk, bk, wv, bv, wp, bp
    )

    key = ("nc", with_qbias)
    if key not in _CACHE:
        _CACHE[key] = _build_nc(with_qbias=with_qbias)
    res = run_bass_kernel_spmd(
        _CACHE[key], in_maps, core_ids=list(range(NCORES)), **_CACHE.get("runkw", {})
    )
    _CACHE["last_result"] = res

    out = np.empty((B, C, N), np.float32)
    for core in range(NCORES):
        bi, hi = core // 2, core % 2
        out[bi, :, hi * NQ : (hi + 1) * NQ] = res.results[core]["y"]
    return out.reshape(B, C, 64, 64)



# revision 9
# speedup vs baseline: 1.0741x; 1.0741x over previous
"""AttentionBlock (GroupNorm + single-head self-attention + residual) on 8 trn2 cores.

Sharding: core = (batch b = core//2, token-half h = core%2).  Each core gets the
full (128, 4096) channel-major image for its batch (needed for groupnorm stats
and full K/V), computes attention only for its 2048-token half, and writes a
(128, 2048) output slab.  The host rolls the token axis per-core so the q-half
is always columns [0:2048] -> one SPMD program for all 8 cores, no collectives.

Algebraic restructure (all exact, folded on the host):
  scoresT[m,n] = k_m . q_n = hn_m^T (Wk^T Wq') hn_n  (+ a per-n constant that
  cancels in softmax; Wq' absorbs the 1/sqrt(c) scale).  With M := Wk^T Wq'
  the score matmuls contract hn tiles against q2 = M.hn -- K is never
  materialized.  attn out: y = xpb + (Wp.Wv).(hn.e)/den with Wv/bv folded into
  W2 = Wp.Wv and xpb = x + bp + Wp.bv precomputed on the host for the q half.

v2 pipeline (fp16 data path, PE/ACT software-pipelined):
  - hn, q2, hnT, attn tiles all fp16: 1 cycle/row on the PE, 2x DVE modes,
    ~2.4e-4 element error (safe: scores here are O(0.1)).
  - flat loop over 44 (qb, k-group) pairs; PE issues scores(g+1) before V(g)
    so the exp(g) on ACT never stalls the PE stream.  ACT is the bottleneck
    engine (65536 exp elements/partition ~ 55us + per-instr overhead).
  - softmax denominator: DVE accumulates exp tiles (fp16, 2x) into a 3-deep
    accumulator; a 3-matmul ones-fold into PSUM finishes it (no per-tile
    ones-matmuls on the PE).
  - PSUM: 2x3-bank scores ring (also hosts the den fold + final 1/den
    broadcast) + 2x1-bank out/proj ring = 8 banks exactly.
  - 1/den partition-broadcast via DRAM DMA bounce, y emission deferred one
    qb to hide it (PE ones-matmul broadcast for the last qb).
"""

import numpy as np

C = 128        # channels
N = 4096       # tokens per batch (64*64)
NQ = 2048      # q tokens per core
B = 4
NCORES = 8
GROUPS = 8
EPS = 1e-5
QB = 512       # q block (one PSUM bank of fp32)
NQB = NQ // QB # 4
KT = 128       # k tile (partition dim)
NKT = N // KT  # 32
KT_GROUPS = [3] * 10 + [2]   # k-tile triples (fewer ACT overheads)
NG = len(KT_GROUPS)

_CACHE = {}


def _build_nc(repeat=1, with_qbias=False):
    from contextlib import ExitStack

    import concourse.bacc as bacc
    import concourse.bass as bass
    import concourse.mybir as mybir
    import concourse.tile as tile
    from concourse.mybir import ActivationFunctionType as AF
    from concourse.mybir import AluOpType as ALU

    fp32 = mybir.dt.float32
    f32r = mybir.dt.float32r
    fp16 = mybir.dt.float16

    nc = bacc.Bacc()

    x_d = nc.dram_tensor("x", [C, N], fp32, kind="ExternalInput")
    xpb_d = nc.dram_tensor("xpb", [C, NQ], fp32, kind="ExternalInput")
    mqk_d = nc.dram_tensor("mqk", [C, C], fp16, kind="ExternalInput")
    w2t_d = nc.dram_tensor("w2t", [C, C], f32r, kind="ExternalInput")
    ident_d = nc.dram_tensor("ident", [C, C], fp16, kind="ExternalInput")
    uq_d = nc.dram_tensor("uq", [C, 1], f32r, kind="ExternalInput")
    gsc_d = nc.dram_tensor("gscale", [C, 1], fp32, kind="ExternalInput")
    gbi_d = nc.dram_tensor("gbias", [C, 1], fp32, kind="ExternalInput")
    mka_d = nc.dram_tensor("maska", [C, GROUPS], fp32, kind="ExternalInput")
    mkb_d = nc.dram_tensor("maskb", [GROUPS, C], fp32, kind="ExternalInput")
    y_d = nc.dram_tensor("y", [C, NQ], fp32, kind="ExternalOutput")

    with tile.TileContext(nc) as tc, ExitStack() as ctx:
        const = ctx.enter_context(tc.tile_pool(name="const", bufs=1))
        big = ctx.enter_context(tc.tile_pool(name="big", bufs=1))
        small = ctx.enter_context(tc.tile_pool(name="small", bufs=1))

        # small consts on the sync queue (ahead of x), matrices on gpsimd's
        gsc_sb = const.tile([C, 1], fp32)
        nc.sync.dma_start(out=gsc_sb, in_=gsc_d[:, :])
        gbi_sb = const.tile([C, 1], fp32)
        nc.sync.dma_start(out=gbi_sb, in_=gbi_d[:, :])
        mka_sb = const.tile([C, GROUPS], fp32)
        nc.sync.dma_start(out=mka_sb, in_=mka_d[:, :])
        mkb_sb = const.tile([GROUPS, C], fp32)
        nc.sync.dma_start(out=mkb_sb, in_=mkb_d[:, :])
        uq_sb = const.tile([C, 1], f32r)
        nc.sync.dma_start(out=uq_sb, in_=uq_d[:, :])
        mqk_sb = const.tile([C, C], fp16)
        nc.gpsimd.dma_start(out=mqk_sb, in_=mqk_d[:, :])
        ident_sb = const.tile([C, C], fp16)
        nc.gpsimd.dma_start(out=ident_sb, in_=ident_d[:, :])
        w2t_sb = const.tile([C, C], f32r)
        nc.gpsimd.dma_start(out=w2t_sb, in_=w2t_d[:, :])

        ones_col = const.tile([C, 1], fp32)
        nc.vector.memset(ones_col, 1.0)
        ones_col_h = const.tile([C, 1], fp16)
        nc.vector.tensor_copy(ones_col_h, ones_col)
        ones_row_r = const.tile([1, QB], f32r)
        nc.vector.memset(ones_row_r.bitcast(mybir.dt.uint32), 0x3F800000)
        ones_rowc_f = const.tile([1, C], fp32)
        nc.vector.memset(ones_rowc_f, 1.0)
        eps_sb = const.tile([C, 1], fp32)
        nc.vector.memset(eps_sb, EPS)
        warm_sb = const.tile([1, 1], fp32)
        nc.scalar.activation(warm_sb, eps_sb[0:1, :], AF.Exp, bias=0.0, scale=0.0)

        rep_ctx = tc.For_i(0, repeat, 1) if repeat > 1 else None
        if rep_ctx is not None:
            rep_ctx.__enter__()

        x_sb = big.tile([C, N], fp32, tag="x")
        for ci in range(8):
            eng = (nc.sync, nc.scalar, nc.gpsimd)[ci % 3]
            eng.dma_start(
                out=x_sb[:, ci * 512 : (ci + 1) * 512],
                in_=x_d[:, ci * 512 : (ci + 1) * 512],
            )
        # xpb is only needed by the deferred y chain -- late is fine
        xpb_sb = big.tile([C, NQ], fp32, tag="xpb")
        for ci in range(4):
            eng = (nc.sync, nc.scalar, nc.gpsimd)[ci % 3]
            eng.dma_start(
                out=xpb_sb[:, ci * 512 : (ci + 1) * 512],
                in_=xpb_d[:, ci * 512 : (ci + 1) * 512],
            )

        # ---- groupnorm stats (+ PE warm-up matmuls in the same pool) ----
        NCHUNK = N // 512
        with tc.tile_pool(name="stat_ps", bufs=2, space="PSUM") as stat_ps:
            # HAM keep-warm: dummy matmuls bridge the PE-idle stats phase so
            # the activity monitor doesn't re-throttle the PE to 1.2GHz.
            # Blips are sequenced via data deps on the arriving stats chunks.
            warm_ps = stat_ps.tile([C, QB], fp32, tag="warm")
            for _ in range(2):
                nc.tensor.matmul(
                    warm_ps[:, 0:C], ident_sb, ident_sb, start=True, stop=True
                )

            stats = small.tile([C, NCHUNK, 6], fp32)
            for i in range(NCHUNK):
                nc.vector.bn_stats(
                    out=stats[:, i, :], in_=x_sb[:, i * 512 : (i + 1) * 512]
                )
                if i in (2, 4, 6):
                    nc.tensor.matmul(
                        warm_ps[0:GROUPS, 0:6], mka_sb, stats[:, i, :],
                        start=True, stop=True,
                    )
            mv = small.tile([C, 2], fp32)
            nc.vector.bn_aggr(out=mv, in_=stats)

            # S = [m, v + m^2] per channel
            S = small.tile([C, 2], fp32)
            nc.vector.tensor_copy(S[:, 0:1], mv[:, 0:1])
            msq = small.tile([C, 1], fp32)
            nc.vector.tensor_mul(msq, mv[:, 0:1], mv[:, 0:1])
            nc.vector.tensor_add(S[:, 1:2], mv[:, 1:2], msq)

            # group-reduce across partitions via mask matmuls
            g_ps = stat_ps.tile([GROUPS, 2], fp32, tag="g")
            nc.tensor.matmul(g_ps, mka_sb, S, start=True, stop=True)
            g_sb = small.tile([GROUPS, 2], fp32)
            nc.vector.tensor_copy(g_sb, g_ps)
            g2_ps = stat_ps.tile([C, 2], fp32, tag="g2")
            nc.tensor.matmul(g2_ps, mkb_sb, g_sb, start=True, stop=True)

            gsz = C // GROUPS
            mean_g = small.tile([C, 1], fp32)
            nc.vector.tensor_scalar_mul(mean_g, g2_ps[:, 0:1], 1.0 / gsz)
            e2_g = small.tile([C, 1], fp32)
            nc.vector.tensor_scalar_mul(e2_g, g2_ps[:, 1:2], 1.0 / gsz)
            var_g = small.tile([C, 1], fp32)
            nc.vector.tensor_mul(var_g, mean_g, mean_g)
            nc.vector.tensor_tensor(out=var_g, in0=e2_g, in1=var_g, op=ALU.subtract)
            # rstd = exp(-0.5*ln(var+eps)) -- stays on the exp table set
            lnv_g = small.tile([C, 1], fp32)
            nc.scalar.activation(lnv_g, var_g, AF.Ln, bias=eps_sb, scale=1.0)
            rstd_g = small.tile([C, 1], fp32)
            nc.scalar.activation(rstd_g, lnv_g, AF.Exp, bias=0.0, scale=-0.5)
            alpha = small.tile([C, 1], fp32)
            nc.vector.tensor_mul(alpha, rstd_g, gsc_sb)
            beta = small.tile([C, 1], fp32)
            nc.vector.tensor_mul(beta, mean_g, alpha)
            nc.vector.tensor_tensor(out=beta, in0=gbi_sb, in1=beta, op=ALU.subtract)

        # ---- hn = alpha*x + beta (DVE, fp16 out) ----
        hn = big.tile([C, N], fp16, tag="hn")
        for ci in range(4):
            nc.vector.tensor_scalar(
                out=hn[:, ci * 1024 : (ci + 1) * 1024],
                in0=x_sb[:, ci * 1024 : (ci + 1) * 1024],
                scalar1=alpha, scalar2=beta, op0=ALU.mult, op1=ALU.add,
            )

        # ---- q2 = M.hn (q half), hnT tiles (full) ----
        q2_sb = big.tile([C, NQB, QB], fp16, tag="q2")
        hnT_sb = big.tile([KT, NKT, C], fp16, tag="hnT")
        gam_sb = None
        if with_qbias:
            gam_sb = big.tile([1, N], f32r, tag="gam")

        with (
            tc.tile_pool(name="qk_ps", bufs=2, space="PSUM") as qk_ps,
            tc.tile_pool(name="v_ps", bufs=2, space="PSUM") as v_ps,
        ):
            # q2 for qb0 first: it gates the first scores matmul
            for j in range(NQB):
                ps2 = qk_ps.tile([C, 512], fp32, tag="qk")
                nc.tensor.matmul(
                    ps2, mqk_sb, hn[:, j * 512 : (j + 1) * 512],
                    start=True, stop=True,
                )
                nc.vector.tensor_copy(q2_sb[:, j, :], ps2)
            for j in range(NQB * 2):
                vp = v_ps.tile([KT, 4, C], fp32, tag="v")
                for t in range(4):
                    kt = j * 4 + t
                    nc.tensor.matmul(
                        vp[:, t, :], hn[:, kt * KT : (kt + 1) * KT], ident_sb,
                        start=True, stop=True,
                    )
                nc.vector.tensor_copy(hnT_sb[:, j * 4 : (j + 1) * 4, :], vp)
            if with_qbias:
                for j in range(N // 512):
                    gp = qk_ps.tile([1, 512], fp32, tag="qg")
                    nc.tensor.matmul(
                        gp, uq_sb, hn[:, j * 512 : (j + 1) * 512],
                        start=True, stop=True,
                    )
                    nc.vector.tensor_copy(
                        gam_sb[:, j * 512 : (j + 1) * 512], gp
                    )

        # ---- attention main loop: flat software pipeline over (qb, group) ----
        aout_sb = big.tile([C, NQB, QB], f32r, tag="aout")
        rden_dram = nc.dram_tensor("rden_scratch", [NQB, QB], fp32, kind="Internal")
        # group boundaries
        g_kt0 = []
        kt = 0
        for gs in KT_GROUPS:
            g_kt0.append(kt)
            kt += gs

        with (
            tc.tile_pool(name="s_ps", bufs=2, space="PSUM") as spool,
            tc.tile_pool(name="o_ps", bufs=2, space="PSUM") as opool,
            tc.tile_pool(name="attn", bufs=3) as apool,
            tc.tile_pool(name="dacc", bufs=2) as dpool,
        ):
            state = {}   # per-qb live tiles
            pending_y = None

            def emit_y(pend):
                pp_sb_, rbc_, qb_ = pend
                y_sb = small.tile([C, QB], fp32, tag="y", bufs=2)
                nc.vector.tensor_mul(y_sb, pp_sb_, rbc_)
                nc.vector.tensor_add(
                    y_sb, y_sb, xpb_sb[:, qb_ * QB : (qb_ + 1) * QB]
                )
                nc.sync.dma_start(out=y_d[:, qb_ * QB : (qb_ + 1) * QB], in_=y_sb)

            def emit_scores(qb, g):
                gsize = KT_GROUPS[g]
                kt0 = g_kt0[g]
                s_ps = spool.tile([KT, 3, QB], fp32, tag="s")
                qv = q2_sb[:, qb, :]
                for t in range(gsize):
                    nc.tensor.matmul(
                        s_ps[:, t, :],
                        hn[:, (kt0 + t) * KT : (kt0 + t + 1) * KT],
                        qv,
                        start=True,
                        stop=(not with_qbias),
                    )
                    if with_qbias:
                        nc.tensor.matmul(
                            s_ps[:, t, :],
                            gam_sb[:, (kt0 + t) * KT : (kt0 + t + 1) * KT],
                            ones_row_r,
                            start=False,
                            stop=True,
                        )
                at = apool.tile([KT, 3, QB], fp16, tag="at")
                nc.scalar.activation(at[:, :gsize, :], s_ps[:, :gsize, :], AF.Exp)
                return at

            def emit_post(qb, g, at):
                # V accumulation for group g (PE) + den accumulation (DVE)
                gsize = KT_GROUPS[g]
                kt0 = g_kt0[g]
                st = state[qb]
                for t in range(gsize):
                    k_idx = kt0 + t
                    nc.tensor.matmul(
                        st["out_ps"],
                        hnT_sb[:, k_idx, :],
                        at[:, t, :],
                        start=(k_idx == 0),
                        stop=(k_idx == NKT - 1),
                    )
                if g == 0:
                    nc.vector.tensor_copy(st["dacc"], at)
                elif g == NG - 1:
                    # last group has 2 tiles; pad add with tile 0 repeated
                    nc.vector.tensor_add(
                        st["dacc"][:, 0:2, :], st["dacc"][:, 0:2, :], at[:, 0:2, :]
                    )
                else:
                    nc.vector.tensor_add(st["dacc"], st["dacc"], at)

            def emit_boundary(qb):
                # everything after the last V matmul of a q-block
                st = state[qb]
                # den fold: 3 accumulating ones-matmuls into a scores-ring slot
                fold_t = spool.tile([KT, 3, QB], fp32, tag="s")
                fold = fold_t[0:1, 0, 0:QB]
                for s in range(3):
                    nc.tensor.matmul(
                        fold, ones_col_h, st["dacc"][:, s, :],
                        start=(s == 0), stop=(s == 2),
                    )
                # unnormalized accumulator out (frees the out bank)
                nc.vector.tensor_copy(aout_sb[:, qb, :], st["out_ps"])
                # reciprocal + partition broadcast
                rden = small.tile([1, QB], fp32, tag="rden", bufs=2)
                nc.vector.reciprocal(rden, fold)
                if qb < NQB - 1:
                    nc.sync.dma_start(out=rden_dram[qb : qb + 1, :], in_=rden)
                    rbc = small.tile([C, QB], fp32, tag="rbc", bufs=2)
                    rd_ap = rden_dram[qb : qb + 1, :]
                    nc.sync.dma_start(
                        out=rbc,
                        in_=bass.AP(
                            tensor=rd_ap.tensor, offset=rd_ap.offset,
                            ap=[[0, C], [1, QB]],
                        ),
                    )
                else:
                    # last qb: PE ones-broadcast into a free scores-ring slot
                    # (fp32 matmul, 4c/row -- tail only, beats a DRAM bounce)
                    rbc_t = spool.tile([KT, 3, QB], fp32, tag="s")
                    rbc = rbc_t[:, 0, 0:QB]
                    nc.tensor.matmul(rbc, ones_rowc_f, rden, start=True, stop=True)
                # projection on the unnormalized accumulator
                pp = opool.tile([C, QB], fp32, tag="o")
                nc.tensor.matmul(
                    pp, w2t_sb, aout_sb[:, qb, :], start=True, stop=True
                )
                pp_sb = small.tile([C, QB], fp32, tag="ppsb", bufs=2)
                nc.vector.tensor_copy(pp_sb, pp)
                return (pp_sb, rbc, qb)

            seq = [(qb, g) for qb in range(NQB) for g in range(NG)]
            prev = None
            for (qb, g) in seq:
                if g == 0:
                    state[qb] = {
                        "out_ps": opool.tile(
                            [C, QB], fp32, tag="o", name="out_ps"
                        ),
                        "dacc": dpool.tile(
                            [KT, 3, QB], fp16, tag="d", name="dacc"
                        ),
                    }
                at = emit_scores(qb, g)
                if prev is not None:
                    pqb, pg, pat = prev
                    emit_post(pqb, pg, pat)
                    if pg == NG - 1:
                        pending_y = emit_boundary(pqb)
                if g == 2 and pending_y is not None:
                    emit_y(pending_y)
                    pending_y = None
                prev = (qb, g, at)

            pqb, pg, pat = prev
            emit_post(pqb, pg, pat)
            pending_y_last = emit_boundary(pqb)
            if pending_y is not None:
                emit_y(pending_y)
            emit_y(pending_y_last)

        if rep_ctx is not None:
            rep_ctx.__exit__(None, None, None)

    nc.compile()
    return nc


def _prep_maps(x):
    x = np.ascontiguousarray(np.asarray(x, dtype=np.float32))
    b, c, h, w = x.shape
    assert (b, c, h * w) == (B, C, N), f"unexpected shape {x.shape}"
    return x.reshape(b, c, h * w)


def _make_in_maps(x, norm_scale, norm_bias, wq, bq, wk, bk, wv, bv, wp, bp):
    xr = _prep_maps(x)
    s = float(C) ** -0.5
    f32 = np.float32
    f64 = np.float64

    wqs = np.asarray(wq, f64) * s
    wk64 = np.asarray(wk, f64)
    wv64 = np.asarray(wv, f64)
    wp64 = np.asarray(wp, f64)
    bq64 = np.asarray(bq, f64) * s
    bv64 = np.asarray(bv, f64)
    bp64 = np.asarray(bp, f64)

    # scores: hn^T (Wk^T Wq') hn ; lhsT for q2 = M.hn is M^T = Wq'^T Wk
    mqk = np.ascontiguousarray((wqs.T @ wk64).astype(np.float16))
    # proj: W2 = Wp.Wv, lhsT = W2^T ; bias bp2 = bp + Wp.bv (folded into xpb)
    w2t = np.ascontiguousarray((wp64 @ wv64).T.astype(f32))
    bp2 = (bp64 + wp64 @ bv64).astype(f32).reshape(C, 1)
    # q-bias term (slow path only): u = Wk^T bq'
    uq = np.ascontiguousarray((wk64.T @ bq64).astype(f32).reshape(C, 1))
    ident = np.ascontiguousarray(np.eye(C, dtype=np.float16))
    gsc = np.ascontiguousarray(np.asarray(norm_scale, f32).reshape(C, 1))
    gbi = np.ascontiguousarray(np.asarray(norm_bias, f32).reshape(C, 1))
    maska = np.zeros((C, GROUPS), f32)
    maska[np.arange(C), np.arange(C) // (C // GROUPS)] = 1.0
    maskb = np.ascontiguousarray(maska.T)

    with_qbias = bool(np.any(np.asarray(bq) != 0))

    in_maps = []
    for core in range(NCORES):
        bi, hi = core // 2, core % 2
        xb = xr[bi]
        if hi:
            xb = np.roll(xb, -NQ, axis=1)
        in_maps.append(
            dict(
                x=np.ascontiguousarray(xb),
                xpb=np.ascontiguousarray(xb[:, :NQ] + bp2),
                mqk=mqk, w2t=w2t, ident=ident, uq=uq,
                gscale=gsc, gbias=gbi, maska=maska, maskb=maskb,
            )
        )
    return in_maps, with_qbias


def kernel(x, norm_scale, norm_bias, wq, bq, wk, bk, wv, bv, wp, bp):
    from concourse.bass_utils import run_bass_kernel_spmd

    in_maps, with_qbias = _make_in_maps(
        x, norm_scale, norm_bias, wq, bq, wk, bk, wv, bv, wp, bp
    )

    key = ("nc", with_qbias)
    if key not in _CACHE:
        _CACHE[key] = _build_nc(with_qbias=with_qbias)
    res = run_bass_kernel_spmd(
        _CACHE[key], in_maps, core_ids=list(range(NCORES)), **_CACHE.get("runkw", {})
    )
    _CACHE["last_result"] = res

    out = np.empty((B, C, N), np.float32)
    for core in range(NCORES):
        bi, hi = core // 2, core % 2
        out[bi, :, hi * NQ : (hi + 1) * NQ] = res.results[core]["y"]
    return out.reshape(B, C, 64, 64)


# revision 22
# speedup vs baseline: 1.2416x; 1.1560x over previous
"""AttentionBlock (GroupNorm + single-head self-attention + residual) on 8 trn2 cores.

Sharding: core = (batch b = core//2, token-half h = core%2).  Each core gets the
full (128, 4096) channel-major image for its batch (needed for groupnorm stats
and full K/V), computes attention only for its 2048-token half, and writes a
(128, 2048) output slab.  The host rolls the token axis per-core so the q-half
is always columns [0:2048] -> one SPMD program for all 8 cores, no collectives.

v4: raw-x attention algebra, fp16 data path, PE/ACT software pipeline.

GroupNorm hn = a*x + b (per-channel a, b from group stats) is never
materialized over the k/v token axis:
  scores:  s[m,n] = hn_m^T M hn_n = x_m^T (a . q2_n) + c_n   (c_n cancels in
           softmax).  q3 = a . (M hn_q) needs hn only for the 2048 q columns.
  attn.V:  out[c,q] = sum_m hn[c,m] at[m,q] = a_c (X at)[c,q] + b_c den[q]
           -> V matmuls contract RAW fp16 x tiles (uploaded as fp16 by the
           host; transposes xT are stats-independent); the b_c den[q] term is
           three PE matmuls of B (rows = (b/a)^T) against the den
           accumulator, and a_c folds into the accumulator copy-out.
rstd = 1/sqrt(var+eps) via DVE-only quake rsqrt + 2 Newton steps: the ACT
engine uses a single table set (Exp) for the whole kernel -- no mid-loop
table reloads, and the one load hides at t=0 behind a warm exp.

Main loop: flat software pipeline over 44 (qb, k-group) units; the PE issues
scores(g+1) before attn.V(g) so ACT exp (the bottleneck: ~1.55us per 3-tile
group) never stalls.  Softmax denominator: fp16 adds into a 3-deep
accumulator (one add per qb offloaded to GpSimd) + 3-matmul ones-fold; 1/den
is partition-broadcast with a PE ones-matmul into a PSUM bank (no DRAM
bounce).  PSUM: 2x3-bank scores ring (also hosts the per-qb projection) +
out + bd = 8 banks.  Big tiles are double-buffered so back-to-back kernel
invocations pipeline across iterations.
"""

import numpy as np

C = 128        # channels
N = 4096       # tokens per batch (64*64)
NQ = 2048      # q tokens per core
B = 4
NCORES = 8
GROUPS = 8
EPS = 1e-5
QB = 512       # q block (one PSUM bank of fp32)
NQB = NQ // QB # 4
KT = 128       # k tile (partition dim)
NKT = N // KT  # 32
KT_GROUPS = [3] * 10 + [2]   # k-tile triples (fewer ACT overheads)
NG = len(KT_GROUPS)
NPK32 = 3 + C + C            # packed fp32 consts: gsc, gbi, uq, Rmat, w2t
QUAKE_MAGIC = 0x5F3759DF

_CACHE = {}


def _build_nc(repeat=1, with_qbias=False):
    from contextlib import ExitStack

    import concourse.bacc as bacc
    import concourse.bass as bass
    import concourse.mybir as mybir
    import concourse.tile as tile
    from concourse.mybir import ActivationFunctionType as AF
    from concourse.mybir import AluOpType as ALU

    fp32 = mybir.dt.float32
    f32r = mybir.dt.float32r
    fp16 = mybir.dt.float16
    u32 = mybir.dt.uint32

    nc = bacc.Bacc()

    xh_d = nc.dram_tensor("xh", [C, N], fp16, kind="ExternalInput")
    xpb_d = nc.dram_tensor("xpb", [C, NQ], fp32, kind="ExternalInput")
    pk32_d = nc.dram_tensor("pk32", [C, NPK32], fp32, kind="ExternalInput")
    pk16_d = nc.dram_tensor("pk16", [C, 2 * C], fp16, kind="ExternalInput")
    y_d = nc.dram_tensor("y", [C, NQ], fp32, kind="ExternalOutput")
    bo_dram = nc.dram_tensor("bo_scratch", [1, C], fp16, kind="Internal")

    with tile.TileContext(nc) as tc, ExitStack() as ctx:
        const = ctx.enter_context(tc.tile_pool(name="const", bufs=1))
        big = ctx.enter_context(tc.tile_pool(name="big", bufs=2))
        small = ctx.enter_context(tc.tile_pool(name="small", bufs=1))

        pk32_sb = const.tile([C, NPK32], fp32)
        nc.gpsimd.dma_start(out=pk32_sb, in_=pk32_d[:, :])
        pk16_sb = const.tile([C, 2 * C], fp16)
        nc.gpsimd.dma_start(out=pk16_sb, in_=pk16_d[:, :])
        gsc = pk32_sb[:, 0:1]
        gbi = pk32_sb[:, 1:2]
        uq_f = pk32_sb[:, 2:3]
        rmat_sb = pk32_sb[:, 3 : 3 + C]
        w2t_sb = const.tile([C, C], f32r)
        nc.vector.tensor_copy(w2t_sb, pk32_sb[:, 3 + C : 3 + 2 * C])
        mqk_sb = pk16_sb[:, 0:C]
        ident_sb = pk16_sb[:, C : 2 * C]

        ones_col_h = const.tile([C, 1], fp16)
        nc.vector.memset(ones_col_h.bitcast(mybir.dt.uint16), 0x3C00)
        ones_row128_h = const.tile([1, C], fp16)
        nc.vector.memset(ones_row128_h.bitcast(mybir.dt.uint16), 0x3C00)
        ones_rowc_r = const.tile([1, C], f32r)
        nc.vector.memset(ones_rowc_r.bitcast(mybir.dt.uint32), 0x3F800000)
        ones_row_h = const.tile([1, QB], fp16)
        nc.vector.memset(ones_row_h.bitcast(mybir.dt.uint16), 0x3C00)
        magic_u = const.tile([C, 1], u32)
        nc.vector.memset(magic_u, QUAKE_MAGIC)
        warm1_sb = const.tile([1, 1], fp32)
        nc.vector.memset(warm1_sb, 1.0)
        # one warm exp: loads the (only) ACT table set at t=0
        nc.scalar.activation(warm1_sb, warm1_sb, AF.Exp, bias=0.0, scale=0.0)

        rep_ctx = tc.For_i(0, repeat, 1) if repeat > 1 else None
        if rep_ctx is not None:
            rep_ctx.__enter__()

        # ---- uploads: xh quarters alternating queues, then xpb halves ----
        xh = big.tile([C, N], fp16, tag="xh")
        for ci in range(4):
            eng = (nc.sync, nc.scalar)[ci % 2]
            eng.dma_start(
                out=xh[:, ci * 1024 : (ci + 1) * 1024],
                in_=xh_d[:, ci * 1024 : (ci + 1) * 1024],
            )
        xpb_sb = big.tile([C, NQ], fp32, tag="xpb")
        nc.sync.dma_start(out=xpb_sb[:, 0:1024], in_=xpb_d[:, 0:1024])
        nc.scalar.dma_start(out=xpb_sb[:, 1024:2048], in_=xpb_d[:, 1024:2048])

        q3_sb = big.tile([C, NQB, QB], fp16, tag="q3")
        xT_sb = big.tile([KT, NKT, C], fp16, tag="xT")
        bmat_sb = big.tile([C, C], fp16, tag="bmat")
        hnq = big.tile([C, NQ], fp16, tag="hnq")
        aout_sb = big.tile([C, NQB, QB], f32r, tag="aout")
        gam_sb = None
        if with_qbias:
            gam_sb = big.tile([1, N], fp16, tag="gam")

        with (
            tc.tile_pool(name="stat_ps", bufs=2, space="PSUM") as stat_ps,
            tc.tile_pool(name="qk_ps", bufs=2, space="PSUM") as qk_ps,
            tc.tile_pool(name="v_ps", bufs=2, space="PSUM") as v_ps,
        ):
            # transposes of raw xh first in the PE stream (xh-gated only);
            # their PSUM->SBUF copies drain on DVE after the stats chain
            xt_pend = []
            for j in range(NKT // 4):
                vp = v_ps.tile([KT, 4, C], fp32, tag="v")
                for t in range(4):
                    kt = j * 4 + t
                    nc.tensor.matmul(
                        vp[:, t, :], xh[:, kt * KT : (kt + 1) * KT], ident_sb,
                        start=True, stop=True,
                    )
                xt_pend.append((j, vp))

            # groupnorm stats
            warm_ps = stat_ps.tile([C, QB], fp32, tag="warm", bufs=1)
            stats = small.tile([C, 8, 6], fp32)
            for i in range(8):
                nc.vector.bn_stats(
                    out=stats[:, i, :], in_=xh[:, i * 512 : (i + 1) * 512]
                )
            mv = small.tile([C, 2], fp32)
            nc.vector.bn_aggr(out=mv, in_=stats)
            # S = [m, v, m^2]; group means via one Rmat matmul (Rmat has
            # 1/group_size in each group block)
            S = small.tile([C, 3], fp32)
            nc.vector.tensor_copy(S[:, 0:2], mv)
            nc.vector.tensor_mul(S[:, 2:3], mv[:, 0:1], mv[:, 0:1])
            g2_ps = stat_ps.tile([C, 3], fp32, tag="g2", bufs=1)
            nc.tensor.matmul(g2_ps, rmat_sb, S, start=True, stop=True)
            g2s = small.tile([C, 3], fp32)
            nc.vector.tensor_copy(g2s, g2_ps)
            ev2 = small.tile([C, 1], fp32)
            nc.vector.tensor_add(ev2, g2s[:, 1:2], g2s[:, 2:3])
            msq = small.tile([C, 1], fp32)
            nc.vector.tensor_mul(msq, g2s[:, 0:1], g2s[:, 0:1])
            vpe = small.tile([C, 1], fp32)
            nc.vector.tensor_scalar(
                out=vpe, in0=ev2, scalar1=msq, scalar2=EPS,
                op0=ALU.subtract, op1=ALU.add,
            )
            # rstd = rsqrt(var+eps): quake initial guess + 2 Newton steps,
            # all on DVE (same-engine chains pipeline fast; keeps ACT on one
            # table set)
            ish = small.tile([C, 1], u32)
            nc.vector.tensor_scalar(
                out=ish, in0=vpe.bitcast(u32), scalar1=1, scalar2=None,
                op0=ALU.logical_shift_right,
            )
            y0u = small.tile([C, 1], u32)
            nc.vector.tensor_tensor(
                out=y0u, in0=magic_u, in1=ish, op=ALU.subtract
            )
            ycur = y0u.bitcast(fp32)
            for it in range(2):
                tmp = small.tile([C, 1], fp32, tag=f"nt{it}", name="tmp")
                nc.vector.tensor_mul(tmp, ycur, ycur)
                nc.vector.tensor_scalar(
                    out=tmp, in0=tmp, scalar1=vpe, scalar2=-0.5,
                    op0=ALU.mult, op1=ALU.mult,
                )
                nc.vector.tensor_scalar(
                    out=tmp, in0=tmp, scalar1=1.5, scalar2=None, op0=ALU.add
                )
                ynew = small.tile([C, 1], fp32, tag=f"ny{it}", name="ynew")
                nc.vector.tensor_mul(ynew, ycur, tmp)
                ycur = ynew
            alpha = small.tile([C, 1], fp32)
            nc.vector.tensor_mul(alpha, ycur, gsc)
            # hn = alpha*x - beta',  beta' = mean*alpha - gbias
            betap = small.tile([C, 1], fp32)
            nc.vector.tensor_scalar(
                out=betap, in0=g2s[:, 0:1], scalar1=alpha, scalar2=gbi,
                op0=ALU.mult, op1=ALU.subtract,
            )
            # HAM keep-warm blip between Rmm and the q2 matmuls
            nc.tensor.matmul(
                warm_ps[:, 0:1], rmat_sb, alpha, start=True, stop=True
            )

            # hn (q half) + q2/q3 per q-block; alpha folds into the copy.
            # q-block 0 first -- it gates the first scores matmul.
            nc.vector.tensor_scalar(
                out=hnq[:, 0:QB], in0=xh[:, 0:QB],
                scalar1=alpha, scalar2=betap, op0=ALU.mult, op1=ALU.subtract,
            )
            ps2_0 = qk_ps.tile([C, 512], fp32, tag="qk", name="ps2")
            nc.tensor.matmul(
                ps2_0, mqk_sb, hnq[:, 0:512], start=True, stop=True
            )
            nc.vector.tensor_scalar_mul(q3_sb[:, 0, :], ps2_0, alpha)
            # xT copies 0-3 (needed by the first V matmuls)
            for j, vp in xt_pend[:4]:
                nc.vector.tensor_copy(xT_sb[:, j * 4 : (j + 1) * 4, :], vp)
            for j in range(1, NQB):
                nc.vector.tensor_scalar(
                    out=hnq[:, j * QB : (j + 1) * QB],
                    in0=xh[:, j * QB : (j + 1) * QB],
                    scalar1=alpha, scalar2=betap,
                    op0=ALU.mult, op1=ALU.subtract,
                )
                ps2 = qk_ps.tile([C, 512], fp32, tag="qk", name="ps2")
                nc.tensor.matmul(
                    ps2, mqk_sb, hnq[:, j * 512 : (j + 1) * 512],
                    start=True, stop=True,
                )
                nc.vector.tensor_scalar_mul(q3_sb[:, j, :], ps2, alpha)
            # remaining xT copies
            for j, vp in xt_pend[4:]:
                nc.vector.tensor_copy(xT_sb[:, j * 4 : (j + 1) * 4, :], vp)
            # Bmat = ones_col . (beta/alpha)^T
            ralpha = small.tile([C, 1], fp32)
            nc.vector.reciprocal(ralpha, alpha)
            bo_h = small.tile([C, 1], fp16)
            nc.vector.tensor_scalar(
                out=bo_h, in0=betap, scalar1=ralpha, scalar2=-1.0,
                op0=ALU.mult, op1=ALU.mult,
            )
            # Bmat rows all equal bo^T: partition-broadcast via DMA bounce
            # (no PE/DVE work; ready well before the first q-block boundary)
            nc.sync.dma_start(out=bo_dram[0:1, :], in_=bo_h)
            bo_ap = bo_dram[0:1, :]
            nc.sync.dma_start(
                out=bmat_sb,
                in_=bass.AP(
                    tensor=bo_ap.tensor, offset=bo_ap.offset, ap=[[0, C], [1, C]]
                ),
            )
            if with_qbias:
                u2_h = small.tile([C, 1], fp16)
                nc.vector.tensor_scalar_mul(u2_h, uq_f, alpha)
                for j in range(N // 512):
                    gp = qk_ps.tile([1, 512], fp32, tag="qg")
                    nc.tensor.matmul(
                        gp, u2_h, xh[:, j * 512 : (j + 1) * 512],
                        start=True, stop=True,
                    )
                    nc.vector.tensor_copy(
                        gam_sb[:, j * 512 : (j + 1) * 512], gp
                    )

        # ---- attention main loop: flat software pipeline over (qb, group) ----
        g_kt0 = []
        kt = 0
        for gs in KT_GROUPS:
            g_kt0.append(kt)
            kt += gs

        with (
            tc.tile_pool(name="s_ps", bufs=2, space="PSUM") as spool,
            tc.tile_pool(name="o_ps", bufs=1, space="PSUM") as opool,
            tc.tile_pool(name="bd_ps", bufs=1, space="PSUM") as bdpool,
            tc.tile_pool(name="attn", bufs=6) as apool,
            tc.tile_pool(name="dacc", bufs=2) as dpool,
        ):
            state = {}
            pending_y = None

            def emit_y(pend):
                pp_sb_, rbc_, qb_ = pend
                y_sb = small.tile([C, QB], fp32, tag="y", bufs=2)
                nc.vector.tensor_mul(y_sb, pp_sb_, rbc_)
                nc.gpsimd.tensor_add(
                    y_sb, y_sb, xpb_sb[:, qb_ * QB : (qb_ + 1) * QB]
                )
                if qb_ == NQB - 1:
                    nc.sync.dma_start(
                        out=y_d[:, qb_ * QB : qb_ * QB + 256],
                        in_=y_sb[:, 0:256],
                    )
                    nc.scalar.dma_start(
                        out=y_d[:, qb_ * QB + 256 : (qb_ + 1) * QB],
                        in_=y_sb[:, 256:512],
                    )
                else:
                    nc.sync.dma_start(
                        out=y_d[:, qb_ * QB : (qb_ + 1) * QB], in_=y_sb
                    )

            def emit_scores(qb, g):
                gsize = KT_GROUPS[g]
                kt0 = g_kt0[g]
                s_ps = spool.tile([KT, 3, QB], fp32, tag="s", name="s_ps")
                qv = q3_sb[:, qb, :]
                for t in range(gsize):
                    nc.tensor.matmul(
                        s_ps[:, t, :],
                        xh[:, (kt0 + t) * KT : (kt0 + t + 1) * KT],
                        qv,
                        start=True,
                        stop=(not with_qbias),
                    )
                    if with_qbias:
                        nc.tensor.matmul(
                            s_ps[:, t, :],
                            gam_sb[:, (kt0 + t) * KT : (kt0 + t + 1) * KT],
                            ones_row_h,
                            start=False,
                            stop=True,
                        )
                at = apool.tile([KT, 3, QB], fp16, tag="at", name="at")
                nc.scalar.activation(at[:, :gsize, :], s_ps[:, :gsize, :], AF.Exp)
                return at

            def emit_post(qb, g, at):
                gsize = KT_GROUPS[g]
                kt0 = g_kt0[g]
                st = state[qb]
                for t in range(gsize):
                    k_idx = kt0 + t
                    nc.tensor.matmul(
                        st["out_ps"],
                        xT_sb[:, k_idx, :],
                        at[:, t, :],
                        start=(k_idx == 0),
                        stop=False,
                    )
                # den accumulation; the initial copy rides the idle GpSimd
                # (done before the g==1 DVE add needs it)
                if g == 0:
                    nc.gpsimd.tensor_copy(st["dacc"], at)
                elif g == NG - 1:
                    nc.vector.tensor_add(
                        st["dacc"][:, 0:2, :], st["dacc"][:, 0:2, :], at[:, 0:2, :]
                    )
                else:
                    nc.vector.tensor_add(st["dacc"], st["dacc"], at)

            def emit_boundary(qb):
                st = state[qb]
                # beta*den correction closes the V accumulation group; alpha
                # folds into the copy-out
                for s in range(3):
                    nc.tensor.matmul(
                        st["out_ps"], bmat_sb, st["dacc"][:, s, :],
                        start=False, stop=(s == 2),
                    )
                nc.vector.tensor_scalar_mul(
                    aout_sb[:, qb, :], st["out_ps"], alpha
                )
                # den fold -> 1/den -> PE partition-broadcast (PSUM-resident)
                bd_t = bdpool.tile([C, QB], fp32, tag="bd", name="bd_t")
                for s in range(3):
                    nc.tensor.matmul(
                        bd_t[0:1, 0:QB], ones_col_h, st["dacc"][:, s, :],
                        start=(s == 0), stop=(s == 2),
                    )
                rden = small.tile([1, QB], f32r, tag="rden", bufs=2)
                with nc.allow_low_precision(
                    reason="f32r is full fp32 bits; only the PE mode is relaxed"
                ):
                    nc.vector.reciprocal(rden, bd_t[0:1, 0:QB])
                nc.tensor.matmul(bd_t, ones_rowc_r, rden, start=True, stop=True)
                # projection reuses the (just copied-out) V-accumulator bank
                pp = opool.tile([C, QB], fp32, tag="o", name="pp_t")
                nc.tensor.matmul(pp, w2t_sb, aout_sb[:, qb, :], start=True, stop=True)
                pp_sb = small.tile([C, QB], fp32, tag="ppsb", bufs=2)
                nc.vector.tensor_copy(pp_sb, pp)
                return (pp_sb, bd_t, qb)

            seq = [(qb, g) for qb in range(NQB) for g in range(NG)]
            prev = None
            for (qb, g) in seq:
                if g == 0:
                    state[qb] = {
                        "out_ps": opool.tile(
                            [C, QB], fp32, tag="o", name="out_ps"
                        ),
                        "dacc": dpool.tile(
                            [KT, 3, QB], fp16, tag="d", name="dacc"
                        ),
                    }
                at = emit_scores(qb, g)
                if prev is not None:
                    pqb, pg, pat = prev
                    emit_post(pqb, pg, pat)
                    if pg == NG - 1:
                        pending_y = emit_boundary(pqb)
                if g == 2 and pending_y is not None:
                    emit_y(pending_y)
                    pending_y = None
                prev = (qb, g, at)

            pqb, pg, pat = prev
            emit_post(pqb, pg, pat)
            pending_y_last = emit_boundary(pqb)
            if pending_y is not None:
                emit_y(pending_y)
            emit_y(pending_y_last)

        if rep_ctx is not None:
            rep_ctx.__exit__(None, None, None)

    nc.compile()
    return nc


def _prep_maps(x):
    x = np.ascontiguousarray(np.asarray(x, dtype=np.float32))
    b, c, h, w = x.shape
    assert (b, c, h * w) == (B, C, N), f"unexpected shape {x.shape}"
    return x.reshape(b, c, h * w)


def _make_in_maps(x, norm_scale, norm_bias, wq, bq, wk, bk, wv, bv, wp, bp):
    xr = _prep_maps(x)
    s = float(C) ** -0.5
    f32 = np.float32
    f64 = np.float64

    wqs = np.asarray(wq, f64) * s
    wk64 = np.asarray(wk, f64)
    wv64 = np.asarray(wv, f64)
    wp64 = np.asarray(wp, f64)
    bq64 = np.asarray(bq, f64) * s
    bv64 = np.asarray(bv, f64)
    bp64 = np.asarray(bp, f64)

    # scores: hn^T (Wk^T Wq') hn ; lhsT for q2 = M.hn is M^T = Wq'^T Wk
    mqk = (wqs.T @ wk64).astype(np.float16)
    # proj: W2 = Wp.Wv, lhsT = W2^T ; bias bp2 = bp + Wp.bv (folded into xpb)
    w2t = (wp64 @ wv64).T.astype(f32)
    bp2 = (bp64 + wp64 @ bv64).astype(f32).reshape(C, 1)
    # q-bias term (slow path only): u = Wk^T bq'
    uq = (wk64.T @ bq64).astype(f32).reshape(C, 1)
    ident = np.eye(C, dtype=np.float16)
    gsc = np.asarray(norm_scale, f32).reshape(C, 1)
    gbi = np.asarray(norm_bias, f32).reshape(C, 1)
    gsz = C // GROUPS
    grp = np.arange(C) // gsz
    rmat = (grp[:, None] == grp[None, :]).astype(f32) / gsz

    pk32 = np.ascontiguousarray(
        np.concatenate([gsc, gbi, uq, rmat, w2t], axis=1)
    )
    assert pk32.shape == (C, NPK32)
    pk16 = np.ascontiguousarray(np.concatenate([mqk, ident], axis=1))

    with_qbias = bool(np.any(np.asarray(bq) != 0))

    in_maps = []
    for core in range(NCORES):
        bi, hi = core // 2, core % 2
        xb = xr[bi]
        if hi:
            xb = np.roll(xb, -NQ, axis=1)
        in_maps.append(
            dict(
                xh=np.ascontiguousarray(xb.astype(np.float16)),
                xpb=np.ascontiguousarray(xb[:, :NQ] + bp2),
                pk32=pk32, pk16=pk16,
            )
        )
    return in_maps, with_qbias


def kernel(x, norm_scale, norm_bias, wq, bq, wk, bk, wv, bv, wp, bp):
    from concourse.bass_utils import run_bass_kernel_spmd

    in_maps, with_qbias = _make_in_maps(
        x, norm_scale, norm_bias, wq, bq, wk, bk, wv, bv, wp, bp
    )

    key = ("nc", with_qbias)
    if key not in _CACHE:
        _CACHE[key] = _build_nc(with_qbias=with_qbias)
    res = run_bass_kernel_spmd(
        _CACHE[key], in_maps, core_ids=list(range(NCORES)), **_CACHE.get("runkw", {})
    )
    _CACHE["last_result"] = res

    out = np.empty((B, C, N), np.float32)
    for core in range(NCORES):
        bi, hi = core // 2, core % 2
        out[bi, :, hi * NQ : (hi + 1) * NQ] = res.results[core]["y"]
    return out.reshape(B, C, 64, 64)
